# revision 1
# baseline (speedup 1.0000x reference)
"""Trainium2 Bass kernel for nn_DCCEngine (cluster-attention pooling block).

Reference computation per batch b:
  sim   = x_flat @ centers.T * C^-0.5        [N,K]   (N=16384 pixels, K=64)
  attn  = softmax(sim, -1)
  cluster = attn.T @ x_flat                  [K,C]
  refined = silu(dwconv7x7(cluster.T as [C,8,8]) + dw_b)
  out   = attn @ refined_flat                [N,C]
  y     = pw_w @ out + pw_b
  result = x + group_norm(y) * gn_g + gn_b

Sharding: pure data-parallel, batch b -> core b (8 cores).

Key structure:
  - softmax without max-subtraction (sim ~ N(0, 0.02^2): exp is safe)
  - per-pixel softmax denominator via an all-ones matmul (J^T @ expT
    replicates the k-sum to all partitions), DVE reciprocal, GPSIMD multiply
  - unpool+pointwise conv fused into one tiny matrix WRT = pw_w @ refined.T
    [256, 64]:  y = WRT @ attnT + pw_b
  - GroupNorm stats computed analytically from the attention Gram matrix
    G = attn^T attn and a = attn^T 1 (no pass over y):
      sum_n y   = WRT @ a + N pw_b
      sum_n y^2 = diag(WRT G WRT^T) + 2 pw_b (WRT@a) + N pw_b^2
  - GN scale folded into WRT rows; final output is one fused DVE op per tile:
    out = (u + const[o]) + x
  - fp32 bits flow through PE in float32r mode (tf32-like mantissa) for the
    big streaming matmuls; PE transposes produce x^T / attn pixel-major.
"""
import numpy as np

import concourse.bass as bass
import concourse.tile as tile
from concourse import bacc, mybir
from concourse.bass_utils import run_bass_kernel_spmd

f32 = mybir.dt.float32
f32r = mybir.dt.float32r

C = 256
CH = 128          # channel half
N = 16384         # pixels per batch
K = 64            # clusters
CHUNK = 512
NCHUNK = N // CHUNK          # 32
GRP = 128                    # pixel group (cluster lhsT tile)
NGRP = CHUNK // GRP          # 4
GROUPS = 32
GSZ = C // GROUPS            # 8 channels per group
EPS = 1e-5

Exp = mybir.ActivationFunctionType.Exp
Sigmoid = mybir.ActivationFunctionType.Sigmoid
Sqrt = mybir.ActivationFunctionType.Sqrt
Copy = mybir.ActivationFunctionType.Copy
MUL = mybir.AluOpType.mult
ADD = mybir.AluOpType.add


def build_nc(repeat: int = 1, sim_f32r: bool = True, bstage: int = 4):
    nc = bacc.Bacc("TRN2", target_bir_lowering=False, debug=False)

    def din(name, shape, dt_=f32):
        return nc.dram_tensor(name, list(shape), dt_, kind="ExternalInput").ap()

    x_d = din("x", [C, N])
    cenT_d = din("cenT", [C, K])       # (centers * C^-0.5).T
    ident_d = din("ident", [CH, CH])
    jmat_d = din("jmat", [K, K])       # all-ones
    ones_d = din("ones", [CH, 2])
    dwt_d = din("dwt", [C, 49])
    dwb_d = din("dwb", [C, 1])
    pwb_d = din("pwb", [C, 1])
    gng_d = din("gng", [C, 1])
    gnb_d = din("gnb", [C, 1])
    pwbN_d = din("pwbN", [C, 1])       # N * pw_b
    npwb2_d = din("npwb2", [C, 1])     # N * pw_b^2
    pwb2_d = din("pwb2", [C, 1])       # 2 * pw_b
    pwT_d = din("pwT", [C, C])         # pw_w.T  ([c, o])
    gind_d = din("gind", [CH, 16])     # channel -> group (within half)
    gindT_d = din("gindT", [16, CH])
    out_d = nc.dram_tensor("out", [C, N], f32, kind="ExternalOutput").ap()

    simdt = f32r if sim_f32r else f32

    def half(ap_, h):
        return ap_[h * CH:(h + 1) * CH, :]

    with tile.TileContext(nc) as tc:
        with (
            tc.tile_pool(name="const", bufs=1) as cp,
            tc.tile_pool(name="xp", bufs=1) as xp,
            tc.tile_pool(name="apool", bufs=1) as apool,
            tc.tile_pool(name="wk", bufs=3) as wk,
            tc.tile_pool(name="wk2", bufs=2) as wk2,
            tc.tile_pool(name="wb", bufs=1) as wb,
        ):
            # ---- constants (all split into channel halves where C-sized) ----
            cenT, dwt, dwb, pwb, gng, gnb, pwbN, npwb2, pwb2, pwTh = (
                [], [], [], [], [], [], [], [], [], [])
            for h in range(2):
                t = cp.tile([CH, K], simdt, name=f"cenT{h}")
                nc.sync.dma_start(
                    t[:], half(cenT_d.bitcast(simdt) if sim_f32r else cenT_d, h))
                cenT.append(t)
                t = cp.tile([CH, 49], f32, name=f"dwt{h}")
                nc.sync.dma_start(t[:], half(dwt_d, h)); dwt.append(t)
                for lst, src in ((dwb, dwb_d), (pwb, pwb_d), (gng, gng_d),
                                 (gnb, gnb_d), (pwbN, pwbN_d),
                                 (npwb2, npwb2_d), (pwb2, pwb2_d)):
                    t = cp.tile([CH, 1], f32, name=f"c{len(lst)}_{id(src) % 997}_{h}")
                    nc.sync.dma_start(t[:], half(src, h)); lst.append(t)
                t = cp.tile([CH, C], f32, name=f"pwT{h}")
                nc.sync.dma_start(t[:], half(pwT_d, h)); pwTh.append(t)
            ident = cp.tile([CH, CH], f32, name="ident")
            nc.sync.dma_start(ident[:], ident_d)
            identr = cp.tile([CH, CH], f32r, name="identr")
            nc.sync.dma_start(identr[:], ident_d.bitcast(f32r))
            identr_hi = cp.tile([CH, K], f32r, name="identr_hi")
            nc.sync.dma_start(identr_hi[K:CH, :], ident_d.bitcast(f32r)[0:K, 0:K])
            j64 = cp.tile([K, K], f32r, name="j64")
            nc.sync.dma_start(j64[:], jmat_d.bitcast(f32r))
            ones = cp.tile([CH, 2], f32r, name="ones")
            nc.sync.dma_start(ones[:], ones_d.bitcast(f32r))
            gind = cp.tile([CH, 16], f32, name="gind")
            nc.sync.dma_start(gind[:], gind_d)
            gindT = cp.tile([16, CH], f32, name="gindT")
            nc.sync.dma_start(gindT[:], gindT_d)

            # ---- persistent big buffers ----
            xh = []
            for h in range(2):
                t = xp.tile([CH, N], f32r, name=f"x{h}")
                xh.append(t)
            # attnT (k-major, normalized): chunks 0..15 -> rows 0:64,
            # chunks 16..31 -> rows 64:128.
            attnT = apool.tile([CH, N // 2], f32r, name="attnT")

            for rep in range(repeat):
                for ch in range(NCHUNK):
                    cs = ch * CHUNK
                    for h in range(2):
                        nc.sync.dma_start(
                            xh[h][:, cs:cs + CHUNK],
                            x_d.bitcast(f32r)[h * CH:(h + 1) * CH, cs:cs + CHUNK])

                # ---------- phase A ----------
                with (
                    tc.tile_pool(name="psA", bufs=2, space="PSUM") as psA,
                    tc.tile_pool(name="psS", bufs=2, space="PSUM") as psS,
                    tc.tile_pool(name="psT", bufs=1, space="PSUM") as psT,
                    tc.tile_pool(name="psX", bufs=1, space="PSUM") as psX,
                    tc.tile_pool(name="psC", bufs=1, space="PSUM") as psC,
                ):
                    clp = psC.tile([K, 322], f32, name="clp", tag="clp")
                    for ch in range(NCHUNK):
                        cs = ch * CHUNK
                        rh = 0 if ch < 16 else K
                        ac = (ch % 16) * CHUNK

                        simp = psA.tile([K, CHUNK], f32, name="simp", tag="simp")
                        nc.tensor.matmul(simp[:], cenT[0][:], xh[0][:, cs:cs + CHUNK],
                                         start=True, stop=False)
                        nc.tensor.matmul(simp[:], cenT[1][:], xh[1][:, cs:cs + CHUNK],
                                         start=False, stop=True)
                        expt = wk.tile([K, CHUNK], f32r, name="expt", tag="expt")
                        nc.scalar.activation(expt[:], simp[:], Exp)
                        srep = psS.tile([K, CHUNK], f32, name="srep", tag="srep")
                        nc.tensor.matmul(srep[:], j64[:], expt[:],
                                         start=True, stop=True)
                        rs = wk.tile([K, CHUNK], f32, name="rs", tag="rs")
                        nc.vector.reciprocal(rs[:], srep[:])
                        nc.gpsimd.tensor_mul(attnT[rh:rh + K, ac:ac + CHUNK],
                                             expt[:].bitcast(f32), rs[:])

                        if bstage < 2:
                            continue
                        att_ps = psT.tile([CH, NGRP * K], f32r, name="att_ps",
                                          tag="att_ps")
                        xt_ps = psX.tile([CH, 1024], f32r, name="xt_ps",
                                         tag="xt_ps")
                        stage = wk.tile([CH, NGRP * 322], f32r, name="stage",
                                        tag="stage")
                        for g in range(NGRP):
                            ps = cs + g * GRP
                            idk = identr[0:K, 0:K] if rh == 0 else identr_hi[K:CH, :]
                            nc.tensor.transpose(
                                att_ps[:, g * K:(g + 1) * K],
                                attnT[rh:rh + K, ac + g * GRP:ac + (g + 1) * GRP],
                                idk)
                            for h in range(2):
                                nc.tensor.transpose(
                                    xt_ps[:, g * 256 + h * CH:g * 256 + (h + 1) * CH],
                                    xh[h][:, ps:ps + GRP], identr[:])
                        stage4 = stage[:].rearrange("p (g c) -> p g c", g=NGRP)
                        nc.vector.tensor_copy(
                            stage4[:, :, 320:322],
                            ones[:].unsqueeze(1).broadcast_to([CH, NGRP, 2]))
                        if ch % 2 == 0:
                            nc.scalar.activation(
                                stage4[:, :, 0:256],
                                xt_ps[:].rearrange("p (g c) -> p g c", g=NGRP),
                                Copy)
                        else:
                            nc.vector.tensor_copy(
                                stage4[:, :, 0:256],
                                xt_ps[:].rearrange("p (g c) -> p g c", g=NGRP))
                        nc.scalar.activation(
                            stage4[:, :, 256:320],
                            att_ps[:].rearrange("p (g c) -> p g c", g=NGRP), Copy)

                        for g in range(NGRP):
                            first = (ch == 0 and g == 0)
                            last = (ch == NCHUNK - 1 and g == NGRP - 1)
                            lhsT = stage[:, g * 322 + 256:g * 322 + 320]
                            nc.tensor.matmul(clp[:], lhsT,
                                             stage[:, g * 322:g * 322 + 322],
                                             start=first, stop=last,
                                             skip_group_check=True)

                    if bstage < 2:
                        pass
                    else:
                        cl_sb = wb.tile([K, 322], f32, name="cl_sb")
                        nc.vector.tensor_copy(cl_sb[:], clp[:])
                if bstage == 1:
                    osb0 = wk2.tile([CH, CHUNK], f32, name="osb", tag="osb")
                    nc.vector.tensor_copy(osb0[:], attnT[:, 0:CHUNK].bitcast(f32))
                    nc.sync.dma_start(out_d[0:CH, 0:CHUNK], osb0[:])
                    continue
                if bstage == 2:
                    osb0 = wk2.tile([K, 322], f32, name="osb2", tag="osb2")
                    nc.vector.tensor_copy(osb0[:], cl_sb[:])
                    nc.sync.dma_start(out_d[0:K, 0:322], osb0[:])
                    continue

                # ---------- phase B ----------
                with tc.tile_pool(name="psB", bufs=1, space="PSUM") as psB:
                    grid = []
                    for h in range(2):
                        gp = psB.tile([CH, K], f32, name=f"gp{h}", tag="gp")
                        nc.tensor.transpose(gp[:], cl_sb[:, h * CH:(h + 1) * CH],
                                            ident[0:K, 0:K])
                        gsb = wb.tile([CH, K], f32, name=f"grid{h}")
                        nc.vector.tensor_copy(gsb[:], gp[:])
                        grid.append(gsb)
                    if bstage == 31:
                        osb0 = wk2.tile([CH, K], f32, name="osb31", tag="osb3")
                        nc.vector.tensor_copy(osb0[:], grid[0][:])
                        nc.sync.dma_start(out_d[0:CH, 0:K], osb0[:])
                        continue
                    refined = []
                    for h in range(2):
                        pad = wb.tile([CH, 196], f32, name=f"pad{h}")
                        nc.vector.memset(pad[:], 0.0)
                        padv = pad[:].rearrange("p (r c) -> p r c", r=14)
                        nc.vector.tensor_copy(
                            padv[:, 3:11, 3:11],
                            grid[h][:].rearrange("p (r c) -> p r c", r=8))
                        acc = wb.tile([CH, K], f32, name=f"racc{h}")
                        nc.vector.memset(acc[:], 0.0)
                        accv = acc[:].rearrange("p (r c) -> p r c", r=8)
                        for t in range(49):
                            dr, dc = t // 7, t % 7
                            nc.vector.scalar_tensor_tensor(
                                accv, padv[:, dr:dr + 8, dc:dc + 8],
                                dwt[h][:, t:t + 1], accv, op0=MUL, op1=ADD)
                        # silu(z) = z * sigmoid(z), z = acc + dw_b
                        sg = wb.tile([CH, K], f32, name=f"sg{h}")
                        nc.scalar.activation(sg[:], acc[:], Sigmoid, bias=dwb[h][:])
                        zt = wb.tile([CH, K], f32, name=f"zt{h}")
                        nc.vector.tensor_scalar_add(zt[:], acc[:], dwb[h][:])
                        ref = wb.tile([CH, K], f32, name=f"ref{h}")
                        nc.vector.tensor_mul(ref[:], zt[:], sg[:])
                        refined.append(ref)
                    if bstage == 32:
                        osb0 = wk2.tile([CH, K], f32, name="osb32", tag="osb3")
                        nc.vector.tensor_copy(osb0[:], refined[0][:])
                        nc.sync.dma_start(out_d[0:CH, 0:K], osb0[:])
                        continue
                    wrt = []
                    for oh in range(2):
                        wp = psB.tile([CH, K], f32, name=f"wp{oh}", tag="wp")
                        for h in range(2):
                            nc.tensor.matmul(
                                wp[:], pwTh[h][:, oh * CH:(oh + 1) * CH],
                                refined[h][:], start=(h == 0), stop=(h == 1))
                        wsb = wb.tile([CH, K], f32, name=f"wrt{oh}")
                        nc.vector.tensor_copy(wsb[:], wp[:])
                        wrt.append(wsb)
                    wrtt = wb.tile([K, C], f32, name="wrtt")
                    for oh in range(2):
                        tp = psB.tile([K, CH], f32, name=f"tp{oh}", tag="tp")
                        nc.tensor.transpose(tp[:], wrt[oh][:], ident[:])
                        nc.vector.tensor_copy(wrtt[:, oh * CH:(oh + 1) * CH], tp[:])
                    if bstage == 33:
                        osb0 = wk2.tile([K, C], f32, name="osb33", tag="osb3")
                        nc.vector.tensor_copy(osb0[:], wrtt[:])
                        nc.sync.dma_start(out_d[0:K, 0:C], osb0[:])
                        continue
                    a_col = cl_sb[:, 320:321]
                    g_mat = cl_sb[:, 256:320]
                    stats = []
                    for oh in range(2):
                        st = wb.tile([CH, 2], f32, name=f"stats{oh}")
                        wa = psB.tile([CH, 1], f32, name=f"wa{oh}", tag="wa")
                        nc.tensor.matmul(wa[:], wrtt[:, oh * CH:(oh + 1) * CH],
                                         a_col, start=True, stop=True)
                        nc.vector.scalar_tensor_tensor(
                            st[:, 0:1], wa[:], 1.0, pwbN[oh][:],
                            op0=MUL, op1=ADD)
                        qp = psB.tile([CH, K], f32, name=f"qp{oh}", tag="qp")
                        nc.tensor.matmul(qp[:], wrtt[:, oh * CH:(oh + 1) * CH],
                                         g_mat, start=True, stop=True)
                        scr = wk2.tile([CH, K], f32, name="scr", tag="scr")
                        quad = wb.tile([CH, 1], f32, name=f"quad{oh}")
                        nc.vector.tensor_mul(scr[:], qp[:], wrt[oh][:])
                        nc.vector.reduce_sum(quad[:], scr[:],
                                             axis=mybir.AxisListType.X)
                        t2 = wb.tile([CH, 1], f32, name=f"t2{oh}")
                        nc.vector.scalar_tensor_tensor(
                            t2[:], wa[:], pwb2[oh][:], npwb2[oh][:],
                            op0=MUL, op1=ADD)
                        nc.vector.tensor_add(st[:, 1:2], t2[:], quad[:])
                        stats.append(st)
                    # gs/mv column-packed: cols [0:2] = half0, [2:4] = half1
                    if bstage == 34:
                        osb0 = wk2.tile([CH, 2], f32, name="osb34", tag="osb3")
                        nc.vector.tensor_copy(osb0[:], stats[0][:])
                        nc.sync.dma_start(out_d[0:CH, 0:2], osb0[:])
                        continue
                    gs = wb.tile([16, 4], f32, name="gs")
                    for oh in range(2):
                        gp2 = psB.tile([16, 2], f32, name=f"gp2{oh}", tag="gp2")
                        nc.tensor.matmul(gp2[:], gind[:], stats[oh][:],
                                         start=True, stop=True)
                        nc.vector.tensor_copy(gs[:, oh * 2:(oh + 1) * 2], gp2[:])
                    gs4 = gs[:].rearrange("p (h c) -> p h c", h=2)
                    mv = wb.tile([16, 4], f32, name="mv")  # [mean, rstd] x half
                    mv4 = mv[:].rearrange("p (h c) -> p h c", h=2)
                    cinv = 1.0 / (GSZ * N)
                    nc.vector.tensor_scalar_mul(mv4[:, :, 0:1], gs4[:, :, 0:1], cinv)
                    ex2 = wb.tile([16, 2], f32, name="ex2")
                    nc.vector.tensor_scalar_mul(ex2[:], gs4[:, :, 1], cinv)
                    m2 = wb.tile([16, 2], f32, name="m2")
                    nc.vector.tensor_mul(m2[:], mv4[:, :, 0], mv4[:, :, 0])
                    var = wb.tile([16, 2], f32, name="var")
                    nc.vector.tensor_sub(var[:], ex2[:], m2[:])
                    epst = wb.tile([16, 1], f32, name="epst")
                    nc.vector.memset(epst[:], EPS)
                    std = wb.tile([16, 2], f32, name="std")
                    nc.scalar.activation(std[:], var[:], Sqrt, bias=epst[:])
                    nc.vector.reciprocal(mv4[:, :, 1], std[:])
                    if bstage == 35:
                        osb0 = wk2.tile([16, 4], f32, name="osb35", tag="osb3")
                        nc.vector.tensor_copy(osb0[:], mv[:])
                        nc.sync.dma_start(out_d[0:16, 0:4], osb0[:])
                        continue
                    lw = wb.tile([CH, C], f32r, name="lw")
                    consts = []
                    for oh in range(2):
                        ep = psB.tile([CH, 2], f32, name=f"ep{oh}", tag="ep")
                        nc.tensor.matmul(ep[:], gindT[:],
                                         mv[:, oh * 2:(oh + 1) * 2],
                                         start=True, stop=True)
                        ach = wb.tile([CH, 1], f32, name=f"ach{oh}")
                        nc.vector.tensor_mul(ach[:], gng[oh][:], ep[:, 1:2])
                        cst = wb.tile([CH, 1], f32, name=f"cst{oh}")
                        nc.vector.tensor_sub(cst[:], pwb[oh][:], ep[:, 0:1])
                        nc.vector.tensor_mul(cst[:], ach[:], cst[:])
                        nc.vector.tensor_add(cst[:], gnb[oh][:], cst[:])
                        consts.append(cst)
                        lwp = wb.tile([CH, K], f32, name=f"lwp{oh}")
                        nc.vector.tensor_scalar_mul(lwp[:], wrt[oh][:], ach[:])
                        ltp = psB.tile([K, CH], f32, name=f"ltp{oh}", tag="ltp")
                        nc.tensor.transpose(ltp[:], lwp[:], ident[:])
                        nc.scalar.activation(lw[0:K, oh * CH:(oh + 1) * CH],
                                             ltp[:], Copy)
                        nc.scalar.activation(lw[K:CH, oh * CH:(oh + 1) * CH],
                                             ltp[:], Copy)

                if bstage == 3:
                    osb0 = wk2.tile([CH, C], f32, name="osb3", tag="osb3")
                    nc.vector.tensor_copy(osb0[:], lw[:].bitcast(f32))
                    nc.sync.dma_start(out_d[0:CH, 0:C], osb0[:])
                    continue
                # ---------- phase C ----------
                with tc.tile_pool(name="psU", bufs=4, space="PSUM") as psU:
                    for ch in range(NCHUNK):
                        cs = ch * CHUNK
                        rh = 0 if ch < 16 else K
                        ac = (ch % 16) * CHUNK
                        for oh in range(2):
                            up = psU.tile([CH, CHUNK], f32, name="up", tag="up")
                            nc.tensor.matmul(up[:],
                                             lw[rh:rh + K, oh * CH:(oh + 1) * CH],
                                             attnT[rh:rh + K, ac:ac + CHUNK],
                                             start=True, stop=True)
                            osb = wk2.tile([CH, CHUNK], f32, name="osb", tag="osb")
                            nc.vector.scalar_tensor_tensor(
                                osb[:], up[:], consts[oh],
                                xh[oh][:, cs:cs + CHUNK].bitcast(f32),
                                op0=ADD, op1=ADD)
                            nc.sync.dma_start(
                                out_d[oh * CH:(oh + 1) * CH, cs:cs + CHUNK],
                                osb[:])
    nc.compile()
    return nc


def host_prep(centers, dw_w, dw_b, pw_w, pw_b, gn_g, gn_b):
    cenT = np.ascontiguousarray((centers * (C ** -0.5)).T.astype(np.float32))
    col = lambda v: np.ascontiguousarray(
        np.asarray(v, dtype=np.float32).reshape(C, 1))
    gind = np.zeros((CH, 16), dtype=np.float32)
    for c in range(CH):
        gind[c, c // GSZ] = 1.0
    return {
        "cenT": cenT,
        "ident": np.eye(CH, dtype=np.float32),
        "jmat": np.ones((K, K), dtype=np.float32),
        "ones": np.ones((CH, 2), dtype=np.float32),
        "dwt": np.ascontiguousarray(
            np.asarray(dw_w, dtype=np.float32).reshape(C, 49)),
        "dwb": col(dw_b), "pwb": col(pw_b), "gng": col(gn_g), "gnb": col(gn_b),
        "pwbN": col(np.asarray(pw_b) * float(N)),
        "npwb2": col(np.asarray(pw_b) * np.asarray(pw_b) * float(N)),
        "pwb2": col(2.0 * np.asarray(pw_b)),
        "pwT": np.ascontiguousarray(np.asarray(pw_w, dtype=np.float32).T),
        "gind": gind,
        "gindT": np.ascontiguousarray(gind.T),
    }


_NC_CACHE = {}


def _get_nc(repeat=1, sim_f32r=True, stage=4):
    key = (repeat, sim_f32r, stage)
    if key not in _NC_CACHE:
        _NC_CACHE[key] = build_nc(repeat=repeat, sim_f32r=sim_f32r, bstage=stage)
    return _NC_CACHE[key]


def kernel(x, centers, dw_w, dw_b, pw_w, pw_b, gn_g, gn_b,
           repeat=1, sim_f32r=True):
    x = np.asarray(x)
    B = x.shape[0]
    nc = _get_nc(repeat=repeat, sim_f32r=sim_f32r)
    consts = host_prep(np.asarray(centers), np.asarray(dw_w), np.asarray(dw_b),
                       np.asarray(pw_w), np.asarray(pw_b),
                       np.asarray(gn_g), np.asarray(gn_b))
    in_maps = []
    for b in range(B):
        m = dict(consts)
        m["x"] = np.ascontiguousarray(x[b].reshape(C, N).astype(np.float32))
        in_maps.append(m)
    res = run_bass_kernel_spmd(nc, in_maps, core_ids=list(range(B)))
    out = np.stack([r["out"].reshape(C, 128, 128) for r in res.results])
    return out.astype(np.float32)



# revision 5
# speedup vs baseline: 210.5138x; 210.5138x over previous
"""Trainium2 Bass kernel for nn_DCCEngine (cluster-attention pooling block).

Reference computation per batch b:
  sim   = x_flat @ centers.T * C^-0.5        [N,K]   (N=16384 pixels, K=64)
  attn  = softmax(sim, -1)
  cluster = attn.T @ x_flat                  [K,C]
  refined = silu(dwconv7x7(cluster.T as [C,8,8]) + dw_b)
  out   = attn @ refined_flat                [N,C]
  y     = pw_w @ out + pw_b
  result = x + group_norm(y) * gn_g + gn_b

Sharding: pure data-parallel, batch b -> core b (8 cores).

Key structure:
  - softmax without max-subtraction (sim ~ N(0, 0.02^2): exp is safe)
  - per-pixel softmax denominator via an all-ones matmul (J^T @ expT
    replicates the k-sum to all partitions), DVE reciprocal, GPSIMD multiply
  - unpool+pointwise conv fused into one tiny matrix WRT = pw_w @ refined.T
    [256, 64]:  y = WRT @ attnT + pw_b
  - GroupNorm stats computed analytically from the attention Gram matrix
    G = attn^T attn and a = attn^T 1 (no pass over y):
      sum_n y   = WRT @ a + N pw_b
      sum_n y^2 = diag(WRT G WRT^T) + 2 pw_b (WRT@a) + N pw_b^2
  - GN scale folded into WRT rows; final output is one fused DVE op per tile:
    out = (u + const[o]) + x
  - fp32 bits flow through PE in float32r mode (tf32-like mantissa) for the
    big streaming matmuls; PE transposes produce x^T / attn pixel-major.
"""
import numpy as np

import concourse.bass as bass
import concourse.tile as tile
from concourse import bacc, mybir
from concourse.bass_utils import run_bass_kernel_spmd

f32 = mybir.dt.float32
f32r = mybir.dt.float32r

C = 256
CH = 128          # channel half
N = 16384         # pixels per batch
K = 64            # clusters
CHUNK = 512
NCHUNK = N // CHUNK          # 32
GRP = 128                    # pixel group (cluster lhsT tile)
NGRP = CHUNK // GRP          # 4
GROUPS = 32
GSZ = C // GROUPS            # 8 channels per group
EPS = 1e-5

Exp = mybir.ActivationFunctionType.Exp
Sigmoid = mybir.ActivationFunctionType.Sigmoid
Sqrt = mybir.ActivationFunctionType.Sqrt
Copy = mybir.ActivationFunctionType.Copy
MUL = mybir.AluOpType.mult
ADD = mybir.AluOpType.add


def build_nc(repeat: int = 1, sim_f32r: bool = True, bstage: int = 4,
             timing: bool = False):
    nc = bacc.Bacc("TRN2", target_bir_lowering=False, debug=False)

    def din(name, shape, dt_=f32):
        return nc.dram_tensor(name, list(shape), dt_, kind="ExternalInput").ap()

    x_d = din("x", [C, N])
    cenT_d = din("cenT", [C, K])       # (centers * C^-0.5).T
    ident_d = din("ident", [CH, CH])
    jmat_d = din("jmat", [K, K])       # all-ones
    ones_d = din("ones", [CH, 2])
    dwt_d = din("dwt", [C, 49])
    dwb_d = din("dwb", [C, 1])
    pwb_d = din("pwb", [C, 1])
    gng_d = din("gng", [C, 1])
    gnb_d = din("gnb", [C, 1])
    pwbN_d = din("pwbN", [C, 1])       # N * pw_b
    npwb2_d = din("npwb2", [C, 1])     # N * pw_b^2
    pwb2_d = din("pwb2", [C, 1])       # 2 * pw_b
    pwT_d = din("pwT", [C, C])         # pw_w.T  ([c, o])
    gind_d = din("gind", [CH, 16])     # channel -> group (within half)
    gindT_d = din("gindT", [16, CH])
    out_kind = "Internal" if timing else "ExternalOutput"
    out_d = nc.dram_tensor("out", [C, N], f32, kind=out_kind).ap()
    tok_d = (nc.dram_tensor("tok", [1, 2], f32, kind="ExternalOutput").ap()
             if timing else None)

    simdt = f32r if sim_f32r else f32

    def half(ap_, h):
        return ap_[h * CH:(h + 1) * CH, :]

    with tile.TileContext(nc) as tc:
        with (
            tc.tile_pool(name="const", bufs=1) as cp,
            tc.tile_pool(name="xp", bufs=1) as xp,
            tc.tile_pool(name="apool", bufs=1) as apool,
            tc.tile_pool(name="wk", bufs=3) as wk,
            tc.tile_pool(name="wk2", bufs=2) as wk2,
            tc.tile_pool(name="wb", bufs=1) as wb,
        ):
            # ---- constants (all split into channel halves where C-sized) ----
            cenT, dwt, dwb, pwb, gng, gnb, pwbN, npwb2, pwb2, pwTh = (
                [], [], [], [], [], [], [], [], [], [])
            for h in range(2):
                t = cp.tile([CH, K], simdt, name=f"cenT{h}")
                nc.sync.dma_start(
                    t[:], half(cenT_d.bitcast(simdt) if sim_f32r else cenT_d, h))
                cenT.append(t)
                t = cp.tile([CH, 49], f32, name=f"dwt{h}")
                nc.sync.dma_start(t[:], half(dwt_d, h)); dwt.append(t)
                for lst, src in ((dwb, dwb_d), (pwb, pwb_d), (gng, gng_d),
                                 (gnb, gnb_d), (pwbN, pwbN_d),
                                 (npwb2, npwb2_d), (pwb2, pwb2_d)):
                    t = cp.tile([CH, 1], f32, name=f"c{len(lst)}_{id(src) % 997}_{h}")
                    nc.sync.dma_start(t[:], half(src, h)); lst.append(t)
                t = cp.tile([CH, C], f32, name=f"pwT{h}")
                nc.sync.dma_start(t[:], half(pwT_d, h)); pwTh.append(t)
            ident = cp.tile([CH, CH], f32, name="ident")
            nc.sync.dma_start(ident[:], ident_d)
            identr = cp.tile([CH, CH], f32r, name="identr")
            nc.sync.dma_start(identr[:], ident_d.bitcast(f32r))
            identr_hi = cp.tile([CH, K], f32r, name="identr_hi")
            nc.sync.dma_start(identr_hi[K:CH, :], ident_d.bitcast(f32r)[0:K, 0:K])
            j64 = cp.tile([K, K], f32r, name="j64")
            nc.sync.dma_start(j64[:], jmat_d.bitcast(f32r))
            ones = cp.tile([CH, 2], f32r, name="ones")
            nc.sync.dma_start(ones[:], ones_d.bitcast(f32r))
            gind = cp.tile([CH, 16], f32, name="gind")
            nc.sync.dma_start(gind[:], gind_d)
            gindT = cp.tile([16, CH], f32, name="gindT")
            nc.sync.dma_start(gindT[:], gindT_d)

            # ---- persistent big buffers ----
            xh = []
            for h in range(2):
                t = xp.tile([CH, N], f32r, name=f"x{h}")
                xh.append(t)
            # attnT (k-major, normalized): chunks 0..15 -> rows 0:64,
            # chunks 16..31 -> rows 64:128.
            attnT = apool.tile([CH, N // 2], f32r, name="attnT")

            for rep in range(repeat):
                for ch in range(NCHUNK):
                    cs = ch * CHUNK
                    for h in range(2):
                        nc.sync.dma_start(
                            xh[h][:, cs:cs + CHUNK],
                            x_d.bitcast(f32r)[h * CH:(h + 1) * CH, cs:cs + CHUNK])

                # ---------- phase A ----------
                with (
                    tc.tile_pool(name="psA", bufs=2, space="PSUM") as psA,
                    tc.tile_pool(name="psS", bufs=2, space="PSUM") as psS,
                    tc.tile_pool(name="psT", bufs=1, space="PSUM") as psT,
                    tc.tile_pool(name="psX", bufs=1, space="PSUM") as psX,
                    tc.tile_pool(name="psC", bufs=1, space="PSUM") as psC,
                ):
                    clp = psC.tile([K, 322], f32, name="clp", tag="clp")
                    for ch in range(NCHUNK):
                        cs = ch * CHUNK
                        rh = 0 if ch < 16 else K
                        ac = (ch % 16) * CHUNK

                        simp = psA.tile([K, CHUNK], f32, name="simp", tag="simp")
                        nc.tensor.matmul(simp[:], cenT[0][:], xh[0][:, cs:cs + CHUNK],
                                         start=True, stop=False)
                        nc.tensor.matmul(simp[:], cenT[1][:], xh[1][:, cs:cs + CHUNK],
                                         start=False, stop=True)
                        expt = wk.tile([K, CHUNK], f32r, name="expt", tag="expt")
                        nc.scalar.activation(expt[:], simp[:], Exp)
                        srep = psS.tile([K, CHUNK], f32, name="srep", tag="srep")
                        nc.tensor.matmul(srep[:], j64[:], expt[:],
                                         start=True, stop=True)
                        rs = wk.tile([K, CHUNK], f32, name="rs", tag="rs")
                        nc.vector.reciprocal(rs[:], srep[:])
                        nc.gpsimd.tensor_mul(attnT[rh:rh + K, ac:ac + CHUNK],
                                             expt[:].bitcast(f32), rs[:])

                        if bstage < 2:
                            continue
                        att_ps = psT.tile([CH, NGRP * K], f32r, name="att_ps",
                                          tag="att_ps")
                        xt_ps = psX.tile([CH, 1024], f32r, name="xt_ps",
                                         tag="xt_ps")
                        stage = wk.tile([CH, NGRP * 322], f32r, name="stage",
                                        tag="stage")
                        for g in range(NGRP):
                            ps = cs + g * GRP
                            idk = identr[0:K, 0:K] if rh == 0 else identr_hi[K:CH, :]
                            nc.tensor.transpose(
                                att_ps[:, g * K:(g + 1) * K],
                                attnT[rh:rh + K, ac + g * GRP:ac + (g + 1) * GRP],
                                idk)
                            for h in range(2):
                                nc.tensor.transpose(
                                    xt_ps[:, g * 256 + h * CH:g * 256 + (h + 1) * CH],
                                    xh[h][:, ps:ps + GRP], identr[:])
                        stage4 = stage[:].rearrange("p (g c) -> p g c", g=NGRP)
                        nc.vector.tensor_copy(
                            stage4[:, :, 320:322],
                            ones[:].unsqueeze(1).broadcast_to([CH, NGRP, 2]))
                        if ch % 2 == 0:
                            nc.scalar.activation(
                                stage4[:, :, 0:256],
                                xt_ps[:].rearrange("p (g c) -> p g c", g=NGRP),
                                Copy)
                        else:
                            nc.vector.tensor_copy(
                                stage4[:, :, 0:256],
                                xt_ps[:].rearrange("p (g c) -> p g c", g=NGRP))
                        nc.scalar.activation(
                            stage4[:, :, 256:320],
                            att_ps[:].rearrange("p (g c) -> p g c", g=NGRP), Copy)

                        for g in range(NGRP):
                            first = (ch == 0 and g == 0)
                            last = (ch == NCHUNK - 1 and g == NGRP - 1)
                            lhsT = stage[:, g * 322 + 256:g * 322 + 320]
                            nc.tensor.matmul(clp[:], lhsT,
                                             stage[:, g * 322:g * 322 + 322],
                                             start=first, stop=last,
                                             skip_group_check=True)

                    if bstage < 2:
                        pass
                    else:
                        cl_sb = wb.tile([K, 322], f32, name="cl_sb")
                        nc.vector.tensor_copy(cl_sb[:], clp[:])
                if bstage == 1:
                    osb0 = wk2.tile([CH, CHUNK], f32, name="osb", tag="osb")
                    nc.vector.tensor_copy(osb0[:], attnT[:, 0:CHUNK].bitcast(f32))
                    nc.sync.dma_start(out_d[0:CH, 0:CHUNK], osb0[:])
                    continue
                if bstage == 2:
                    osb0 = wk2.tile([K, 322], f32, name="osb2", tag="osb2")
                    nc.vector.tensor_copy(osb0[:], cl_sb[:])
                    nc.sync.dma_start(out_d[0:K, 0:322], osb0[:])
                    continue

                # ---------- phase B ----------
                with tc.tile_pool(name="psB", bufs=1, space="PSUM") as psB:
                    grid = []
                    for h in range(2):
                        gp = psB.tile([CH, K], f32, name=f"gp{h}", tag="gp")
                        nc.tensor.transpose(gp[:], cl_sb[:, h * CH:(h + 1) * CH],
                                            ident[0:K, 0:K])
                        gsb = wb.tile([CH, K], f32, name=f"grid{h}")
                        nc.vector.tensor_copy(gsb[:], gp[:])
                        grid.append(gsb)
                    if bstage == 31:
                        osb0 = wk2.tile([CH, K], f32, name="osb31", tag="osb3")
                        nc.vector.tensor_copy(osb0[:], grid[0][:])
                        nc.sync.dma_start(out_d[0:CH, 0:K], osb0[:])
                        continue
                    refined = []
                    for h in range(2):
                        pad = wb.tile([CH, 196], f32, name=f"pad{h}")
                        nc.vector.memset(pad[:], 0.0)
                        padv = pad[:].rearrange("p (r c) -> p r c", r=14)
                        nc.vector.tensor_copy(
                            padv[:, 3:11, 3:11],
                            grid[h][:].rearrange("p (r c) -> p r c", r=8))
                        acc = wb.tile([CH, K], f32, name=f"racc{h}")
                        nc.vector.memset(acc[:], 0.0)
                        accv = acc[:].rearrange("p (r c) -> p r c", r=8)
                        for t in range(49):
                            dr, dc = t // 7, t % 7
                            nc.vector.scalar_tensor_tensor(
                                accv, padv[:, dr:dr + 8, dc:dc + 8],
                                dwt[h][:, t:t + 1], accv, op0=MUL, op1=ADD)
                        # silu(z) = z * sigmoid(z), z = acc + dw_b
                        sg = wb.tile([CH, K], f32, name=f"sg{h}")
                        nc.scalar.activation(sg[:], acc[:], Sigmoid, bias=dwb[h][:])
                        zt = wb.tile([CH, K], f32, name=f"zt{h}")
                        nc.vector.tensor_scalar_add(zt[:], acc[:], dwb[h][:])
                        ref = wb.tile([CH, K], f32, name=f"ref{h}")
                        nc.vector.tensor_mul(ref[:], zt[:], sg[:])
                        refined.append(ref)
                    if bstage == 32:
                        osb0 = wk2.tile([CH, K], f32, name="osb32", tag="osb3")
                        nc.vector.tensor_copy(osb0[:], refined[0][:])
                        nc.sync.dma_start(out_d[0:CH, 0:K], osb0[:])
                        continue
                    wrt = []
                    for oh in range(2):
                        wp = psB.tile([CH, K], f32, name=f"wp{oh}", tag="wp")
                        for h in range(2):
                            nc.tensor.matmul(
                                wp[:], pwTh[h][:, oh * CH:(oh + 1) * CH],
                                refined[h][:], start=(h == 0), stop=(h == 1))
                        wsb = wb.tile([CH, K], f32, name=f"wrt{oh}")
                        nc.vector.tensor_copy(wsb[:], wp[:])
                        wrt.append(wsb)
                    wrtt = wb.tile([K, C], f32, name="wrtt")
                    for oh in range(2):
                        tp = psB.tile([K, CH], f32, name=f"tp{oh}", tag="tp")
                        nc.tensor.transpose(tp[:], wrt[oh][:], ident[:])
                        nc.vector.tensor_copy(wrtt[:, oh * CH:(oh + 1) * CH], tp[:])
                    if bstage == 33:
                        osb0 = wk2.tile([K, C], f32, name="osb33", tag="osb3")
                        nc.vector.tensor_copy(osb0[:], wrtt[:])
                        nc.sync.dma_start(out_d[0:K, 0:C], osb0[:])
                        continue
                    a_col = cl_sb[:, 320:321]
                    g_mat = cl_sb[:, 256:320]
                    stats = []
                    for oh in range(2):
                        st = wb.tile([CH, 2], f32, name=f"stats{oh}")
                        wa = psB.tile([CH, 1], f32, name=f"wa{oh}", tag="wa")
                        nc.tensor.matmul(wa[:], wrtt[:, oh * CH:(oh + 1) * CH],
                                         a_col, start=True, stop=True)
                        nc.vector.scalar_tensor_tensor(
                            st[:, 0:1], wa[:], 1.0, pwbN[oh][:],
                            op0=MUL, op1=ADD)
                        qp = psB.tile([CH, K], f32, name=f"qp{oh}", tag="qp")
                        nc.tensor.matmul(qp[:], wrtt[:, oh * CH:(oh + 1) * CH],
                                         g_mat, start=True, stop=True)
                        scr = wk2.tile([CH, K], f32, name="scr", tag="scr")
                        quad = wb.tile([CH, 1], f32, name=f"quad{oh}")
                        nc.vector.tensor_mul(scr[:], qp[:], wrt[oh][:])
                        nc.vector.reduce_sum(quad[:], scr[:],
                                             axis=mybir.AxisListType.X)
                        t2 = wb.tile([CH, 1], f32, name=f"t2{oh}")
                        nc.vector.scalar_tensor_tensor(
                            t2[:], wa[:], pwb2[oh][:], npwb2[oh][:],
                            op0=MUL, op1=ADD)
                        nc.vector.tensor_add(st[:, 1:2], t2[:], quad[:])
                        stats.append(st)
                    # gs/mv column-packed: cols [0:2] = half0, [2:4] = half1
                    if bstage == 34:
                        osb0 = wk2.tile([CH, 2], f32, name="osb34", tag="osb3")
                        nc.vector.tensor_copy(osb0[:], stats[0][:])
                        nc.sync.dma_start(out_d[0:CH, 0:2], osb0[:])
                        continue
                    gs = wb.tile([16, 4], f32, name="gs")
                    for oh in range(2):
                        gp2 = psB.tile([16, 2], f32, name=f"gp2{oh}", tag="gp2")
                        nc.tensor.matmul(gp2[:], gind[:], stats[oh][:],
                                         start=True, stop=True)
                        nc.vector.tensor_copy(gs[:, oh * 2:(oh + 1) * 2], gp2[:])
                    gs4 = gs[:].rearrange("p (h c) -> p h c", h=2)
                    mv = wb.tile([16, 4], f32, name="mv")  # [mean, rstd] x half
                    mv4 = mv[:].rearrange("p (h c) -> p h c", h=2)
                    cinv = 1.0 / (GSZ * N)
                    nc.vector.tensor_scalar_mul(mv4[:, :, 0:1], gs4[:, :, 0:1], cinv)
                    ex2 = wb.tile([16, 2], f32, name="ex2")
                    nc.vector.tensor_scalar_mul(ex2[:], gs4[:, :, 1], cinv)
                    m2 = wb.tile([16, 2], f32, name="m2")
                    nc.vector.tensor_mul(m2[:], mv4[:, :, 0], mv4[:, :, 0])
                    var = wb.tile([16, 2], f32, name="var")
                    nc.vector.tensor_sub(var[:], ex2[:], m2[:])
                    epst = wb.tile([16, 1], f32, name="epst")
                    nc.vector.memset(epst[:], EPS)
                    std = wb.tile([16, 2], f32, name="std")
                    nc.scalar.activation(std[:], var[:], Sqrt, bias=epst[:])
                    nc.vector.reciprocal(mv4[:, :, 1], std[:])
                    if bstage == 35:
                        osb0 = wk2.tile([16, 4], f32, name="osb35", tag="osb3")
                        nc.vector.tensor_copy(osb0[:], mv[:])
                        nc.sync.dma_start(out_d[0:16, 0:4], osb0[:])
                        continue
                    lw = wb.tile([CH, C], f32r, name="lw")
                    consts = []
                    for oh in range(2):
                        ep = psB.tile([CH, 2], f32, name=f"ep{oh}", tag="ep")
                        nc.tensor.matmul(ep[:], gindT[:],
                                         mv[:, oh * 2:(oh + 1) * 2],
                                         start=True, stop=True)
                        ach = wb.tile([CH, 1], f32, name=f"ach{oh}")
                        nc.vector.tensor_mul(ach[:], gng[oh][:], ep[:, 1:2])
                        cst = wb.tile([CH, 1], f32, name=f"cst{oh}")
                        nc.vector.tensor_sub(cst[:], pwb[oh][:], ep[:, 0:1])
                        nc.vector.tensor_mul(cst[:], ach[:], cst[:])
                        nc.vector.tensor_add(cst[:], gnb[oh][:], cst[:])
                        consts.append(cst)
                        lwp = wb.tile([CH, K], f32, name=f"lwp{oh}")
                        nc.vector.tensor_scalar_mul(lwp[:], wrt[oh][:], ach[:])
                        ltp = psB.tile([K, CH], f32, name=f"ltp{oh}", tag="ltp")
                        nc.tensor.transpose(ltp[:], lwp[:], ident[:])
                        nc.scalar.activation(lw[0:K, oh * CH:(oh + 1) * CH],
                                             ltp[:], Copy)
                        nc.scalar.activation(lw[K:CH, oh * CH:(oh + 1) * CH],
                                             ltp[:], Copy)

                if bstage == 3:
                    osb0 = wk2.tile([CH, C], f32, name="osb3", tag="osb3")
                    nc.vector.tensor_copy(osb0[:], lw[:].bitcast(f32))
                    nc.sync.dma_start(out_d[0:CH, 0:C], osb0[:])
                    continue
                # ---------- phase C ----------
                with tc.tile_pool(name="psU", bufs=4, space="PSUM") as psU:
                    for ch in range(NCHUNK):
                        cs = ch * CHUNK
                        rh = 0 if ch < 16 else K
                        ac = (ch % 16) * CHUNK
                        for oh in range(2):
                            up = psU.tile([CH, CHUNK], f32, name="up", tag="up")
                            nc.tensor.matmul(up[:],
                                             lw[rh:rh + K, oh * CH:(oh + 1) * CH],
                                             attnT[rh:rh + K, ac:ac + CHUNK],
                                             start=True, stop=True)
                            osb = wk2.tile([CH, CHUNK], f32, name="osb", tag="osb")
                            nc.vector.scalar_tensor_tensor(
                                osb[:], up[:], consts[oh],
                                xh[oh][:, cs:cs + CHUNK].bitcast(f32),
                                op0=ADD, op1=ADD)
                            nc.sync.dma_start(
                                out_d[oh * CH:(oh + 1) * CH, cs:cs + CHUNK],
                                osb[:])
            if timing:
                tk = wb.tile([1, 2], f32, name="tok")
                nc.vector.memset(tk[:], 1.0)
                nc.sync.dma_start(tok_d, tk[:])
    nc.compile()
    return nc


def host_prep(centers, dw_w, dw_b, pw_w, pw_b, gn_g, gn_b):
    cenT = np.ascontiguousarray((centers * (C ** -0.5)).T.astype(np.float32))
    col = lambda v: np.ascontiguousarray(
        np.asarray(v, dtype=np.float32).reshape(C, 1))
    gind = np.zeros((CH, 16), dtype=np.float32)
    for c in range(CH):
        gind[c, c // GSZ] = 1.0
    return {
        "cenT": cenT,
        "ident": np.eye(CH, dtype=np.float32),
        "jmat": np.ones((K, K), dtype=np.float32),
        "ones": np.ones((CH, 2), dtype=np.float32),
        "dwt": np.ascontiguousarray(
            np.asarray(dw_w, dtype=np.float32).reshape(C, 49)),
        "dwb": col(dw_b), "pwb": col(pw_b), "gng": col(gn_g), "gnb": col(gn_b),
        "pwbN": col(np.asarray(pw_b) * float(N)),
        "npwb2": col(np.asarray(pw_b) * np.asarray(pw_b) * float(N)),
        "pwb2": col(2.0 * np.asarray(pw_b)),
        "pwT": np.ascontiguousarray(np.asarray(pw_w, dtype=np.float32).T),
        "gind": gind,
        "gindT": np.ascontiguousarray(gind.T),
    }


_NC_CACHE = {}


def _get_nc(repeat=1, sim_f32r=True, stage=4, timing=False):
    key = (repeat, sim_f32r, stage, timing)
    if key not in _NC_CACHE:
        _NC_CACHE[key] = build_nc(repeat=repeat, sim_f32r=sim_f32r,
                                  bstage=stage, timing=timing)
    return _NC_CACHE[key]


def kernel(x, centers, dw_w, dw_b, pw_w, pw_b, gn_g, gn_b,
           repeat=1, sim_f32r=True):
    x = np.asarray(x)
    B = x.shape[0]
    nc = _get_nc(repeat=repeat, sim_f32r=sim_f32r)
    consts = host_prep(np.asarray(centers), np.asarray(dw_w), np.asarray(dw_b),
                       np.asarray(pw_w), np.asarray(pw_b),
                       np.asarray(gn_g), np.asarray(gn_b))
    in_maps = []
    for b in range(B):
        m = dict(consts)
        m["x"] = np.ascontiguousarray(x[b].reshape(C, N).astype(np.float32))
        in_maps.append(m)
    res = run_bass_kernel_spmd(nc, in_maps, core_ids=list(range(B)))
    out = np.stack([r["out"].reshape(C, 128, 128) for r in res.results])
    return out.astype(np.float32)



# revision 20
# speedup vs baseline: 381.4235x; 1.8119x over previous
"""Trainium2 Bass kernel for nn_DCCEngine (cluster-attention pooling block).

Reference computation per batch b:
  sim   = x_flat @ centers.T * C^-0.5        [N,K]   (N=16384 pixels, K=64)
  attn  = softmax(sim, -1)
  cluster = attn.T @ x_flat                  [K,C]
  refined = silu(dwconv7x7(cluster.T as [C,8,8]) + dw_b)
  out   = attn @ refined_flat                [N,C]
  y     = pw_w @ out + pw_b
  result = x + group_norm(y) * gn_g + gn_b

Sharding: pure data-parallel, batch b -> core b (8 cores).

Key structure:
  - softmax without max-subtraction (sim ~ N(0, 0.02^2): exp is safe)
  - per-pixel softmax denominator via an all-ones matmul (J^T @ expT
    replicates the k-sum to all partitions), DVE reciprocal, GPSIMD multiply
  - unpool+pointwise conv fused into one tiny matrix WRT = pw_w @ refined.T
    [256, 64]:  y = WRT @ attnT + pw_b
  - GroupNorm stats computed analytically from the attention Gram matrix
    G = attn^T attn and a = attn^T 1 (no pass over y):
      sum_n y   = WRT @ a + N pw_b
      sum_n y^2 = diag(WRT G WRT^T) + 2 pw_b (WRT@a) + N pw_b^2
  - GN scale folded into WRT rows; final output is one fused DVE op per tile:
    out = (u + const[o]) + x, computed IN PLACE over the x tile and stored
    with grouped (4-chunk) DMAs
  - all constants ride in a single packed [128, 1090] DMA; x loads in 4
    big group DMAs (HWDGE descriptor-issue is ~625ns per DMA instruction,
    so DMA count matters more than size)
  - fp32 bits flow through PE in float32r mode (tf32-like mantissa) for the
    big streaming matmuls; PE transposes produce x^T / attn pixel-major.
"""
import numpy as np

import concourse.bass as bass
import concourse.tile as tile
from concourse import bacc, mybir
from concourse.bass_utils import run_bass_kernel_spmd

f32 = mybir.dt.float32
f32r = mybir.dt.float32r

C = 256
CH = 128          # channel half
N = 16384         # pixels per batch
K = 64            # clusters
CHUNK = 512
NCHUNK = N // CHUNK          # 32
GRP = 128                    # pixel group (cluster lhsT tile)
NGRP = CHUNK // GRP          # 4
GROUPS = 32
GSZ = C // GROUPS            # 8 channels per group
EPS = 1e-5
LG = 4096                    # x load group (8 chunks)
SGRP = 4                     # store group (chunks per output DMA)

# packed-constant column offsets (cpk [128, CC] f32).
# Columns 0:NR are consumed by fp32r matmuls and ride in an f32r-tagged DMA;
# the rest load as plain f32.
O_CEN = 0          # 2 x 64
O_ID = 128         # 128 (I_128)
O_J = 256          # 64 (ones [64,64], rows 0:64)
O_ONES = 320       # 2
NR = 322           # end of f32r section
O_DWT = 322        # 2 x 49
O_COL = 420        # 14 single columns
O_PWT = 434        # 2 x 256
O_GI = 946         # 16
O_GIT = 962        # 128 (rows 0:16)
CC = 1090

Exp = mybir.ActivationFunctionType.Exp
Sigmoid = mybir.ActivationFunctionType.Sigmoid
Sqrt = mybir.ActivationFunctionType.Sqrt
Copy = mybir.ActivationFunctionType.Copy
Identity = mybir.ActivationFunctionType.Identity
MUL = mybir.AluOpType.mult
ADD = mybir.AluOpType.add


def build_nc(repeat: int = 1, sim_f32r: bool = True, bstage: int = 4,
             timing: bool = False):
    nc = bacc.Bacc("TRN2", target_bir_lowering=False, debug=False)

    simdt = f32r if sim_f32r else f32

    x_d = nc.dram_tensor("x", [C, N], f32, kind="ExternalInput").ap()
    cpk_d = nc.dram_tensor("cpk", [CH, CC], f32, kind="ExternalInput").ap()
    out_kind = "Internal" if timing else "ExternalOutput"
    out_d = nc.dram_tensor("out", [C, N], f32, kind=out_kind).ap()
    tok_d = (nc.dram_tensor("tok", [1, 2], f32, kind="ExternalOutput").ap()
             if timing else None)

    with tile.TileContext(nc) as tc:
        with (
            tc.tile_pool(name="const", bufs=1) as cp,
            tc.tile_pool(name="xp", bufs=1) as xp,
            tc.tile_pool(name="apool", bufs=1) as apool,
            tc.tile_pool(name="wk", bufs=3) as wk,
            tc.tile_pool(name="wk2", bufs=2) as wk2,
            tc.tile_pool(name="wb", bufs=1) as wb,
        ):
            # ---- constants: two packed DMAs (f32r + f32 tiles) ----
            cpr = cp.tile([CH, NR], f32r, name="cpr")
            nc.sync.dma_start(cpr[:], cpk_d.bitcast(f32r)[:, 0:NR])
            cpf = cp.tile([CH, CC - NR], f32, name="cpf")
            nc.sync.dma_start(cpf[:], cpk_d[:, NR:CC])
            cen = [cpr[:, O_CEN + h * K:O_CEN + (h + 1) * K] for h in range(2)]
            if not sim_f32r:
                cen = [a.bitcast(f32) for a in cen]
            dwt = [cpf[:, O_DWT - NR + 49 * h:O_DWT - NR + 49 * (h + 1)]
                   for h in range(2)]

            def ccol(i):
                return cpf[:, O_COL - NR + i:O_COL - NR + i + 1]

            dwb = [ccol(h) for h in range(2)]
            pwb = [ccol(2 + h) for h in range(2)]
            gng = [ccol(4 + h) for h in range(2)]
            gnb = [ccol(6 + h) for h in range(2)]
            pwbN = [ccol(8 + h) for h in range(2)]
            npwb2 = [ccol(10 + h) for h in range(2)]
            pwb2 = [ccol(12 + h) for h in range(2)]
            pwTh = [cpf[:, O_PWT - NR + C * h:O_PWT - NR + C * (h + 1)]
                    for h in range(2)]
            identr = cpr[:, O_ID:O_ID + CH]
            ident = identr.bitcast(f32)
            # I_64 sitting on partitions 64:128 (rows/cols 64:128 of I_128)
            id64hi = cpr[K:CH, O_ID + K:O_ID + CH]
            j64 = cpr[0:K, O_J:O_J + K]
            ones = cpr[:, O_ONES:O_ONES + 2]
            gind = cpf[:, O_GI - NR:O_GI - NR + 16]
            gindT = cpf[0:16, O_GIT - NR:O_GIT - NR + CH]

            # ---- persistent big buffers ----
            xh = [xp.tile([CH, N], f32r, name=f"x{h}") for h in range(2)]
            # attnT (k-major, normalized): chunks 0..15 -> rows 0:64,
            # chunks 16..31 -> rows 64:128.
            attnT = apool.tile([CH, N // 2], f32r, name="attnT")

            for rep in range(repeat):
                for g in range(N // LG):
                    for h in range(2):
                        nc.sync.dma_start(
                            xh[h][:, g * LG:(g + 1) * LG],
                            x_d.bitcast(f32r)[h * CH:(h + 1) * CH,
                                              g * LG:(g + 1) * LG])

                # ---------- phase A ----------
                # PSUM budget (8 banks): simp 1 + srep 1 + att_ps 1 +
                # xt_ps 2x2 + clp 1.  The cluster matmul runs one chunk
                # behind (on the previous stage tile) so PE never waits on
                # the PSUM->SBUF stage copies.
                with (
                    tc.tile_pool(name="psA", bufs=1, space="PSUM") as psA,
                    tc.tile_pool(name="psS", bufs=1, space="PSUM") as psS,
                    tc.tile_pool(name="psT", bufs=1, space="PSUM") as psT,
                    tc.tile_pool(name="psX", bufs=2, space="PSUM") as psX,
                    tc.tile_pool(name="psC", bufs=1, space="PSUM") as psC,
                ):
                    clp = psC.tile([K, 322], f32, name="clp", tag="clp")

                    def cluster_acc(st, ch):
                        for g in range(NGRP):
                            first = (ch == 0 and g == 0)
                            last = (ch == NCHUNK - 1 and g == NGRP - 1)
                            lhsT = st[:, g * 322 + 256:g * 322 + 320]
                            nc.tensor.matmul(clp[:], lhsT,
                                             st[:, g * 322:g * 322 + 322],
                                             start=first, stop=last,
                                             skip_group_check=True)

                    # Two-deep software pipeline: in slot t the PE runs
                    # sim(t), xT(t), srep(t), attT(t-1), cluster(t-2).  The
                    # softmax chain (exp->srep->recip->mul) of chunk t
                    # resolves during slot t+1, so attT(t) never stalls PE;
                    # stage copies of chunk t finish during slot t+1, so
                    # cluster(t) never stalls PE.
                    stages = {}

                    def softmax_part(ch):
                        cs = ch * CHUNK
                        rh = 0 if ch < 16 else K
                        ac = (ch % 16) * CHUNK
                        simp = psA.tile([K, CHUNK], f32, name="simp", tag="simp")
                        nc.tensor.matmul(simp[:], cen[0],
                                         xh[0][:, cs:cs + CHUNK],
                                         start=True, stop=False)
                        nc.tensor.matmul(simp[:], cen[1],
                                         xh[1][:, cs:cs + CHUNK],
                                         start=False, stop=True)
                        expt = wk.tile([K, CHUNK], f32r, name="expt", tag="expt",
                                       bufs=2)
                        nc.scalar.activation(expt[:], simp[:], Exp)

                        xt_ps = psX.tile([CH, 1024], f32r, name="xt_ps",
                                         tag="xt_ps")
                        stage = wk.tile([CH, NGRP * 322], f32r, name="stage",
                                        tag="stage", bufs=4)
                        stages[ch] = stage
                        for g in range(NGRP):
                            ps = cs + g * GRP
                            for h in range(2):
                                nc.tensor.transpose(
                                    xt_ps[:, g * 256 + h * CH:g * 256 + (h + 1) * CH],
                                    xh[h][:, ps:ps + GRP],
                                    identr)
                        srep = psS.tile([K, CHUNK], f32, name="srep", tag="srep")
                        nc.tensor.matmul(srep[:], j64, expt[:],
                                         start=True, stop=True)
                        rs = wk.tile([K, CHUNK], f32, name="rs", tag="rs",
                                     bufs=2)
                        nc.vector.reciprocal(rs[:], srep[:])
                        nc.gpsimd.tensor_mul(attnT[rh:rh + K, ac:ac + CHUNK],
                                             expt[:].bitcast(f32), rs[:])

                        stage4 = stage[:].rearrange("p (g c) -> p g c", g=NGRP)
                        nc.gpsimd.tensor_copy(
                            stage4[:, :, 320:322],
                            ones.unsqueeze(1).broadcast_to([CH, NGRP, 2]))
                        nc.vector.tensor_copy(
                            stage4[:, :, 0:256],
                            xt_ps[:].rearrange("p (g c) -> p g c", g=NGRP))

                    def att_part(ch):
                        rh = 0 if ch < 16 else K
                        ac = (ch % 16) * CHUNK
                        att_ps = psT.tile([CH, NGRP * K], f32r, name="att_ps",
                                          tag="att_ps")
                        for g in range(NGRP):
                            idk = identr[0:K, 0:K] if rh == 0 else id64hi
                            nc.tensor.transpose(
                                att_ps[:, g * K:(g + 1) * K],
                                attnT[rh:rh + K, ac + g * GRP:ac + (g + 1) * GRP],
                                idk)
                        stage4 = stages[ch][:].rearrange("p (g c) -> p g c",
                                                         g=NGRP)
                        nc.scalar.activation(
                            stage4[:, :, 256:320],
                            att_ps[:].rearrange("p (g c) -> p g c", g=NGRP), Copy)

                    for t in range(NCHUNK + 3):
                        if t < NCHUNK:
                            softmax_part(t)
                        if 0 <= t - 2 < NCHUNK:
                            att_part(t - 2)
                        if t >= 3:
                            cluster_acc(stages.pop(t - 3), t - 3)

                    cl_sb = wb.tile([K, 322], f32, name="cl_sb")
                    nc.vector.tensor_copy(cl_sb[:], clp[:])

                # ---------- phase B ----------
                with tc.tile_pool(name="psB", bufs=1, space="PSUM") as psB:
                    grid = []
                    for h in range(2):
                        gp = psB.tile([CH, K], f32, name=f"gp{h}", tag="gp")
                        nc.tensor.transpose(gp[:], cl_sb[:, h * CH:(h + 1) * CH],
                                            ident[0:K, 0:K])
                        gsb = wb.tile([CH, K], f32, name=f"grid{h}")
                        nc.vector.tensor_copy(gsb[:], gp[:])
                        grid.append(gsb)
                    refined = []
                    for h in range(2):
                        # taps run on DVE (TensorScalarPtr is DVE-only);
                        # Pool handles the cheap pad/memset/silu pieces of
                        # the second half so the halves overlap a little
                        eng = nc.vector if h == 0 else nc.gpsimd
                        pad = wb.tile([CH, 196], f32, name=f"pad{h}")
                        eng.memset(pad[:], 0.0)
                        padv = pad[:].rearrange("p (r c) -> p r c", r=14)
                        eng.tensor_copy(
                            padv[:, 3:11, 3:11],
                            grid[h][:].rearrange("p (r c) -> p r c", r=8))
                        acc = wb.tile([CH, K], f32, name=f"racc{h}")
                        eng.memset(acc[:], 0.0)
                        accv = acc[:].rearrange("p (r c) -> p r c", r=8)
                        for t in range(49):
                            dr, dc = t // 7, t % 7
                            nc.vector.scalar_tensor_tensor(
                                accv, padv[:, dr:dr + 8, dc:dc + 8],
                                dwt[h][:, t:t + 1], accv, op0=MUL, op1=ADD)
                        # silu(z) = z * sigmoid(z), z = acc + dw_b
                        sg = wb.tile([CH, K], f32, name=f"sg{h}")
                        nc.scalar.activation(sg[:], acc[:], Sigmoid, bias=dwb[h])
                        zt = wb.tile([CH, K], f32, name=f"zt{h}")
                        eng.tensor_scalar_add(zt[:], acc[:], dwb[h])
                        ref = wb.tile([CH, K], f32, name=f"ref{h}")
                        eng.tensor_mul(ref[:], zt[:], sg[:])
                        refined.append(ref)
                    wrt = []
                    for oh in range(2):
                        wp = psB.tile([CH, K], f32, name=f"wp{oh}", tag="wp")
                        for h in range(2):
                            nc.tensor.matmul(
                                wp[:], pwTh[h][:, oh * CH:(oh + 1) * CH],
                                refined[h][:], start=(h == 0), stop=(h == 1))
                        wsb = wb.tile([CH, K], f32, name=f"wrt{oh}")
                        nc.vector.tensor_copy(wsb[:], wp[:])
                        wrt.append(wsb)
                    wrtt = wb.tile([K, C], f32, name="wrtt")
                    for oh in range(2):
                        tp = psB.tile([K, CH], f32, name=f"tp{oh}", tag="tp")
                        nc.tensor.transpose(tp[:], wrt[oh][:], ident)
                        nc.vector.tensor_copy(wrtt[:, oh * CH:(oh + 1) * CH], tp[:])
                    a_col = cl_sb[:, 320:321]
                    g_mat = cl_sb[:, 256:320]
                    stats = []
                    for oh in range(2):
                        st = wb.tile([CH, 2], f32, name=f"stats{oh}")
                        wa = psB.tile([CH, 1], f32, name=f"wa{oh}", tag="wa")
                        nc.tensor.matmul(wa[:], wrtt[:, oh * CH:(oh + 1) * CH],
                                         a_col, start=True, stop=True)
                        nc.vector.scalar_tensor_tensor(
                            st[:, 0:1], wa[:], 1.0, pwbN[oh],
                            op0=MUL, op1=ADD)
                        qp = psB.tile([CH, K], f32, name=f"qp{oh}", tag="qp")
                        nc.tensor.matmul(qp[:], wrtt[:, oh * CH:(oh + 1) * CH],
                                         g_mat, start=True, stop=True)
                        scr = wk2.tile([CH, K], f32, name="scr", tag="scr")
                        quad = wb.tile([CH, 1], f32, name=f"quad{oh}")
                        nc.vector.tensor_mul(scr[:], qp[:], wrt[oh][:])
                        nc.vector.reduce_sum(quad[:], scr[:],
                                             axis=mybir.AxisListType.X)
                        t2 = wb.tile([CH, 1], f32, name=f"t2{oh}")
                        nc.vector.scalar_tensor_tensor(
                            t2[:], wa[:], pwb2[oh], npwb2[oh],
                            op0=MUL, op1=ADD)
                        nc.vector.tensor_add(st[:, 1:2], t2[:], quad[:])
                        stats.append(st)
                    gs = wb.tile([16, 4], f32, name="gs")
                    for oh in range(2):
                        gp2 = psB.tile([16, 2], f32, name=f"gp2{oh}", tag="gp2")
                        nc.tensor.matmul(gp2[:], gind, stats[oh][:],
                                         start=True, stop=True)
                        nc.vector.tensor_copy(gs[:, oh * 2:(oh + 1) * 2], gp2[:])
                    gs4 = gs[:].rearrange("p (h c) -> p h c", h=2)
                    mv = wb.tile([16, 4], f32, name="mv")  # [mean, rstd] x half
                    mv4 = mv[:].rearrange("p (h c) -> p h c", h=2)
                    cinv = 1.0 / (GSZ * N)
                    nc.vector.tensor_scalar_mul(mv4[:, :, 0:1], gs4[:, :, 0:1], cinv)
                    ex2 = wb.tile([16, 2], f32, name="ex2")
                    nc.vector.tensor_scalar_mul(ex2[:], gs4[:, :, 1], cinv)
                    m2 = wb.tile([16, 2], f32, name="m2")
                    nc.vector.tensor_mul(m2[:], mv4[:, :, 0], mv4[:, :, 0])
                    var = wb.tile([16, 2], f32, name="var")
                    nc.vector.tensor_sub(var[:], ex2[:], m2[:])
                    epst = wb.tile([16, 1], f32, name="epst")
                    nc.vector.memset(epst[:], EPS)
                    std = wb.tile([16, 2], f32, name="std")
                    nc.scalar.activation(std[:], var[:], Sqrt, bias=epst[:])
                    nc.vector.reciprocal(mv4[:, :, 1], std[:])
                    lw = wb.tile([CH, C], f32r, name="lw")
                    consts = []
                    for oh in range(2):
                        ep = psB.tile([CH, 2], f32, name=f"ep{oh}", tag="ep")
                        nc.tensor.matmul(ep[:], gindT,
                                         mv[:, oh * 2:(oh + 1) * 2],
                                         start=True, stop=True)
                        ach = wb.tile([CH, 1], f32, name=f"ach{oh}")
                        nc.vector.tensor_mul(ach[:], gng[oh], ep[:, 1:2])
                        cst = wb.tile([CH, 1], f32, name=f"cst{oh}")
                        nc.vector.tensor_sub(cst[:], pwb[oh], ep[:, 0:1])
                        nc.vector.tensor_mul(cst[:], ach[:], cst[:])
                        nc.vector.tensor_add(cst[:], gnb[oh], cst[:])
                        consts.append(cst)
                        lwp = wb.tile([CH, K], f32, name=f"lwp{oh}")
                        nc.vector.tensor_scalar_mul(lwp[:], wrt[oh][:], ach[:])
                        ltp = psB.tile([K, CH], f32, name=f"ltp{oh}", tag="ltp")
                        nc.tensor.transpose(ltp[:], lwp[:], ident)
                        nc.scalar.activation(lw[0:K, oh * CH:(oh + 1) * CH],
                                             ltp[:], Copy)
                        nc.scalar.activation(lw[K:CH, oh * CH:(oh + 1) * CH],
                                             ltp[:], Copy)

                # ---------- phase C ----------
                # In-place: xh <- (up + const) + xh, then grouped stores.
                with tc.tile_pool(name="psU", bufs=8, space="PSUM") as psU:
                    for sg_ in range(NCHUNK // SGRP):
                        for ci in range(SGRP):
                            ch = sg_ * SGRP + ci
                            cs = ch * CHUNK
                            rh = 0 if ch < 16 else K
                            ac = (ch % 16) * CHUNK
                            for oh in range(2):
                                up = psU.tile([CH, CHUNK], f32, name="up",
                                              tag="up")
                                nc.tensor.matmul(
                                    up[:],
                                    lw[rh:rh + K, oh * CH:(oh + 1) * CH],
                                    attnT[rh:rh + K, ac:ac + CHUNK],
                                    start=True, stop=True)
                                xv = xh[oh][:, cs:cs + CHUNK]
                                if oh == 0:
                                    nc.vector.scalar_tensor_tensor(
                                        xv, up[:], consts[oh], xv.bitcast(f32),
                                        op0=ADD, op1=ADD)
                                else:
                                    # Pool has no PSUM port: Act moves
                                    # (up + const) to SBUF, Pool adds x
                                    ub = wk2.tile([CH, CHUNK], f32,
                                                  name="ub", tag="ub")
                                    nc.scalar.activation(ub[:], up[:],
                                                         Identity,
                                                         bias=consts[oh])
                                    nc.gpsimd.tensor_add(
                                        xv, ub[:], xv.bitcast(f32))
                        gs_ = sg_ * SGRP * CHUNK
                        for oh in range(2):
                            nc.sync.dma_start(
                                out_d[oh * CH:(oh + 1) * CH,
                                      gs_:gs_ + SGRP * CHUNK],
                                xh[oh][:, gs_:gs_ + SGRP * CHUNK].bitcast(f32))
            if timing:
                tk = wb.tile([1, 2], f32, name="tok")
                nc.vector.memset(tk[:], 1.0)
                nc.sync.dma_start(tok_d, tk[:])
    nc.compile()
    return nc


def host_prep(centers, dw_w, dw_b, pw_w, pw_b, gn_g, gn_b):
    cpk = np.zeros((CH, CC), dtype=np.float32)
    cenT = (np.asarray(centers, dtype=np.float32) * (C ** -0.5)).T  # [C, K]
    for h in range(2):
        cpk[:, O_CEN + h * K:O_CEN + (h + 1) * K] = cenT[h * CH:(h + 1) * CH]
    dwt = np.asarray(dw_w, dtype=np.float32).reshape(C, 49)
    for h in range(2):
        cpk[:, O_DWT + 49 * h:O_DWT + 49 * (h + 1)] = dwt[h * CH:(h + 1) * CH]

    def col(v):
        return np.asarray(v, dtype=np.float32).reshape(C)

    cols = [col(dw_b), col(pw_b), col(gn_g), col(gn_b),
            col(pw_b) * float(N), col(pw_b) * col(pw_b) * float(N),
            2.0 * col(pw_b)]
    for i, v in enumerate(cols):
        for h in range(2):
            cpk[:, O_COL + 2 * i + h] = v[h * CH:(h + 1) * CH]
    pwT = np.asarray(pw_w, dtype=np.float32).T  # [c, o]
    for h in range(2):
        cpk[:, O_PWT + C * h:O_PWT + C * (h + 1)] = pwT[h * CH:(h + 1) * CH]
    cpk[:, O_ID:O_ID + CH] = np.eye(CH, dtype=np.float32)
    cpk[0:K, O_J:O_J + K] = 1.0
    cpk[:, O_ONES:O_ONES + 2] = 1.0
    gind = np.zeros((CH, 16), dtype=np.float32)
    for c_ in range(CH):
        gind[c_, c_ // GSZ] = 1.0
    cpk[:, O_GI:O_GI + 16] = gind
    cpk[0:16, O_GIT:O_GIT + CH] = gind.T
    return {"cpk": np.ascontiguousarray(cpk)}


_NC_CACHE = {}


def _get_nc(repeat=1, sim_f32r=True, stage=4, timing=False):
    key = (repeat, sim_f32r, stage, timing)
    if key not in _NC_CACHE:
        _NC_CACHE[key] = build_nc(repeat=repeat, sim_f32r=sim_f32r,
                                  bstage=stage, timing=timing)
    return _NC_CACHE[key]


def kernel(x, centers, dw_w, dw_b, pw_w, pw_b, gn_g, gn_b,
           repeat=1, sim_f32r=True):
    x = np.asarray(x)
    B = x.shape[0]
    nc = _get_nc(repeat=repeat, sim_f32r=sim_f32r)
    consts = host_prep(np.asarray(centers), np.asarray(dw_w), np.asarray(dw_b),
                       np.asarray(pw_w), np.asarray(pw_b),
                       np.asarray(gn_g), np.asarray(gn_b))
    in_maps = []
    for b in range(B):
        m = dict(consts)
        m["x"] = np.ascontiguousarray(x[b].reshape(C, N).astype(np.float32))
        in_maps.append(m)
    res = run_bass_kernel_spmd(nc, in_maps, core_ids=list(range(B)))
    out = np.stack([r["out"].reshape(C, 128, 128) for r in res.results])
    return out.astype(np.float32)
